# revision 24
# baseline (speedup 1.0000x reference)
"""Adaptive average pooling (8,384,384,64) NHWC -> (8,7,7,64) on 8 TRN2 NeuronCores.

Pure data parallel: one batch sample per core, no collectives. Per core the
full sample lands in SBUF as bf16 once (no ring; only 5 boundary columns
are re-read):

  - W is covered by spans [328,384) [0,110) [109,220) [219,275) [274,330)
    streamed in that order as 15 SWDGE slabs (span x 3 H-chunks) that
    cast f32 -> bf16 in flight, alternating two SWDGE queues. Windows
    are processed [6,0,1,2,3,4,5] to match arrival order, so only one
    window's work (w5) remains when the stream ends; the final slab
    (= window 5's 56-col view at k2) arrives as 7 chunk-DMAs aligned to
    the 512-element matmul chunks so its stop-matmuls + reductions
    pipeline with the arrival. Wide (110-col) slabs are used where two
    windows share a span: 28KB descriptor rows split evenly across the
    16 SDMA engines.
  - TensorEngine reduces over H (partition dim): per (window, H-chunk k)
    the stationary P_k (128 x 7) is a 0/1 bf16 membership mask of the
    H-windows; the moving operand is a contiguous 512-column slice of
    the window's 56-wide view, accumulating psum[i, w', c] over k into
    bank cb.
  - DVE reduces each PSUM bank over w' right after its stop-matmul into
    its own part_sb slice, then combines the 7 partials per window. The
    out-of-window column of the two 55-wide windows (0 and 6) is
    excluded by slicing the reduce view -- no copy/subtract. DVE runs
    its two ports concurrently, so ops that reuse a buffer are ordered
    with explicit self-waits.
  - ACT applies the exact fp32 1/(sh_i*sw_j) scale as a per-partition
    activation scale and DMAs each window's 64-channel column out on
    its own HWDGE ring as soon as it is scaled.

Raw Bass blocks with explicit semaphores.
"""

import numpy as np
import ml_dtypes

import concourse.bass as bass
import concourse.mybir as mybir
from concourse.bass_utils import run_bass_kernel_spmd

B, H, W, C = 8, 384, 384, 64
OUT = 7
N_CORES = 8
KH = H // 128  # 3 H-chunks of 128 rows
NCH = 7  # 512-col matmul chunks per window

# SWDGE spans in stream order; window 6's span first.
SPANS = [(328, 384), (0, 110), (109, 220), (219, 275), (274, 330)]
SOFF = [0, 56, 166, 277, 333]  # span start offset (cols) within xbf's per-k block
XCOLS = 389  # total cols stored per H-chunk
# window j -> (span index, col offset of its 56-wide view within the span)
WIN_SRC = [(1, 0), (1, 54), (2, 0), (2, 55), (3, 0), (4, 0), (0, 0)]
WORDER = [6, 0, 1, 2, 3, 4, 5]  # processing order == arrival order

_F32 = mybir.dt.float32
_BF16 = mybir.dt.bfloat16


def _part_slice(j, cb):
    """(psum col offset, n cols, n w's) for the DVE partial reduce of (j, cb);
    windows 0 and 6 are 55 wide: drop the garbage w' by shortening the slice."""
    if j == 0 and cb == NCH - 1:
        return cb * 512, 448, 7  # drop view col 55 (w'=7 of cb6)
    if j == OUT - 1 and cb == 0:
        return 64, 448, 7  # drop view col 0 (w'=0 of cb0)
    return cb * 512, 512, 8


def _windows(d, out):
    starts = np.floor(np.arange(out) * d / out).astype(np.int64)
    ends = np.ceil((np.arange(out) + 1) * d / out).astype(np.int64)
    return starts, ends - starts


def _build():
    nc = bass.Bass(num_swdge_queues=2)
    x = nc.declare_dram_parameter("x", [H, W * C], _F32, isOutput=False)
    pmat16 = nc.declare_dram_parameter("pmat16", [128, KH * OUT], _BF16, isOutput=False)
    inv = nc.declare_dram_parameter("inv", [OUT, OUT], _F32, isOutput=False)
    out = nc.declare_dram_parameter("out", [OUT, OUT * C], _F32, isOutput=True)

    with (
        nc.sbuf_tensor([128, KH * XCOLS * C], _BF16) as xbf,
        nc.sbuf_tensor([128, KH * OUT], _BF16) as p16_sb,
        nc.sbuf_tensor([OUT, OUT], _F32) as inv_sb,
        nc.sbuf_tensor([OUT, NCH * C], _F32) as part_sb,
        nc.sbuf_tensor([OUT, OUT * C], _F32) as y_raw,
        nc.sbuf_tensor([OUT, OUT * C], _F32) as y_sb,
        nc.psum_tensor([128, NCH * 512], _F32) as psum,
        nc.semaphore("const_sem") as const_sem,
        nc.semaphore("chunk_sem") as chunk_sem,
        nc.semaphore("part_sem") as part_sem,
        nc.semaphore("dve_sem") as dve_sem,
        nc.semaphore("act_sem") as act_sem,
        nc.semaphore("out_sem") as out_sem,
    ):
        g_sems = {
            (g, k): nc.alloc_semaphore(f"g{g}_{k}")
            for g in range(len(SPANS))
            for k in range(KH)
            if not (g == len(SPANS) - 1 and k == KH - 1)
        }
        last_sems = [nc.alloc_semaphore(f"last{cb}") for cb in range(NCH)]

        with nc.Block() as block:

            @block.gpsimd
            def _(gpsimd):
                q = 0
                for g, (a, b) in enumerate(SPANS):
                    for k in range(KH):
                        if g == 0 and k == 0:
                            # split the first slab across both queues so all
                            # 16 SDMA engines have work immediately
                            mid = (a + b) // 2
                            for p, (pa, pb) in enumerate(((a, mid), (mid, b))):
                                dma = gpsimd.dma_start(
                                    out=xbf[
                                        :,
                                        k * XCOLS * C
                                        + (SOFF[g] + pa - a) * C : k * XCOLS * C
                                        + (SOFF[g] + pb - a) * C,
                                    ],
                                    in_=x[k * 128 : (k + 1) * 128, pa * C : pb * C],
                                ).then_inc(g_sems[(g, k)], 16)
                                if q:
                                    dma.ins.queue = "qPoolDynamic1"
                                q ^= 1
                            continue
                        if g == len(SPANS) - 1 and k == KH - 1:
                            # final slab = window 5's view: 7 chunk-DMAs so
                            # its tail pipelines with arrival
                            for cb in range(NCH):
                                ca = a + 8 * cb
                                dma = gpsimd.dma_start(
                                    out=xbf[
                                        :,
                                        k * XCOLS * C
                                        + (SOFF[g] + 8 * cb) * C : k * XCOLS * C
                                        + (SOFF[g] + 8 * (cb + 1)) * C,
                                    ],
                                    in_=x[
                                        k * 128 : (k + 1) * 128,
                                        ca * C : (ca + 8) * C,
                                    ],
                                ).then_inc(last_sems[cb], 16)
                                if q:
                                    dma.ins.queue = "qPoolDynamic1"
                                q ^= 1
                            continue
                        dma = gpsimd.dma_start(
                            out=xbf[
                                :,
                                k * XCOLS * C
                                + SOFF[g] * C : k * XCOLS * C
                                + (SOFF[g] + b - a) * C,
                            ],
                            in_=x[k * 128 : (k + 1) * 128, a * C : b * C],
                        ).then_inc(g_sems[(g, k)], 16)
                        if q:
                            dma.ins.queue = "qPoolDynamic1"
                        q ^= 1

            @block.sync
            def _(sync):
                sync.dma_start(out=p16_sb[:], in_=pmat16[:]).then_inc(const_sem, 16)
                sync.dma_start(out=inv_sb[:], in_=inv[:]).then_inc(const_sem, 16)

            @block.scalar
            def _(scalar):
                scalar.wait_ge(const_sem, 32)
                for o, j in enumerate(WORDER):
                    scalar.wait_ge(dve_sem, o + 1)
                    scalar.activation(
                        y_sb[:, j * C : (j + 1) * C],
                        y_raw[:, j * C : (j + 1) * C],
                        mybir.ActivationFunctionType.Copy,
                        bias=0.0,
                        scale=inv_sb[:, j : j + 1],
                    ).then_inc(act_sem, 1)
                    scalar.wait_ge(act_sem, o + 1)
                    scalar.dma_start(
                        out=out[:, j * C : (j + 1) * C],
                        in_=y_sb[:, j * C : (j + 1) * C],
                    ).then_inc(out_sem, 16)
                scalar.wait_ge(out_sem, OUT * 16)

            @block.tensor
            def _(tensor):
                tensor.wait_ge(const_sem, 16)
                for o, j in enumerate(WORDER):
                    g, off = WIN_SRC[j]
                    for k in range(KH):
                        if (g, k) in g_sems:
                            # the first slab is split into two DMAs
                            tensor.wait_ge(g_sems[(g, k)], 32 if (g, k) == (0, 0) else 16)
                        if o > 0 and k == 0:
                            # WAR: all of the previous window's banks reduced
                            # (single wait -- per-bank counts are unsafe
                            # because DVE partials can complete out of order)
                            tensor.wait_ge(part_sem, o * NCH)
                        base = k * XCOLS * C + (SOFF[g] + off) * C
                        lhsT = p16_sb[:, k * OUT : (k + 1) * OUT]
                        for cb in range(NCH):
                            if g == len(SPANS) - 1 and k == KH - 1:
                                tensor.wait_ge(last_sems[cb], 16)
                            mm = tensor.matmul(
                                psum[:OUT, cb * 512 : (cb + 1) * 512],
                                lhsT,
                                xbf[:, base + cb * 512 : base + (cb + 1) * 512],
                                start=(k == 0),
                                stop=(k == KH - 1),
                            )
                            if k == KH - 1:
                                mm.then_inc(chunk_sem, 1)

            @block.vector
            def _(vector):
                for o, j in enumerate(WORDER):
                    for cb in range(NCH):
                        if o > 0 and cb == 0:
                            # self-wait: the previous window's combine must
                            # have read part_sb before we overwrite it (DVE
                            # runs its two ports concurrently)
                            vector.wait_ge(dve_sem, o)
                        vector.wait_ge(chunk_sem, o * NCH + cb + 1)
                        lo, n, wn = _part_slice(j, cb)
                        vector.tensor_reduce(
                            out=part_sb[:, cb * C : (cb + 1) * C],
                            in_=psum[:OUT, lo : lo + n].rearrange(
                                "p (w c) -> p c w", c=C
                            ),
                            axis=mybir.AxisListType.X,
                            op=mybir.AluOpType.add,
                        ).then_inc(part_sem, 1)
                    vector.wait_ge(part_sem, (o + 1) * NCH)
                    vector.tensor_reduce(
                        out=y_raw[:, j * C : (j + 1) * C],
                        in_=part_sb[:].rearrange("p (n c) -> p c n", c=C),
                        axis=mybir.AxisListType.X,
                        op=mybir.AluOpType.add,
                    ).then_inc(dve_sem, 1)

    return nc


def _consts():
    hs, hsz = _windows(H, OUT)
    _, wsz = _windows(W, OUT)
    p = np.zeros((128, KH * OUT), np.float32)
    for k in range(KH):
        for i in range(OUT):
            h0, h1 = int(hs[i]), int(hs[i] + hsz[i])
            for h in range(max(h0, k * 128), min(h1, (k + 1) * 128)):
                p[h - k * 128, k * OUT + i] = 1.0
    inv = np.zeros((OUT, OUT), np.float32)
    for i in range(OUT):
        for j in range(OUT):
            inv[i, j] = 1.0 / (float(hsz[i]) * float(wsz[j]))
    return p.astype(ml_dtypes.bfloat16), inv


_NC_CACHE = None


def _run(x, **kwargs):
    global _NC_CACHE
    if _NC_CACHE is None:
        _NC_CACHE = _build()
    nc = _NC_CACHE
    p16, inv = _consts()
    x = np.ascontiguousarray(np.asarray(x, dtype=np.float32))
    in_maps = [
        {"x": x[b].reshape(H, W * C), "pmat16": p16, "inv": inv}
        for b in range(N_CORES)
    ]
    res = run_bass_kernel_spmd(nc, in_maps, core_ids=list(range(N_CORES)), **kwargs)
    y = np.stack(
        [res.results[b]["out"].reshape(OUT, OUT, C) for b in range(N_CORES)]
    )
    return y, res


def kernel(x: np.ndarray) -> np.ndarray:
    y, _ = _run(x)
    return y
